# revision 1
# baseline (speedup 1.0000x reference)
"""Trainium2 Bass kernel for AdaNSABlock (7x7 neighborhood attention block).

Sharding: 8 cores = batch(4) x row-halves(2). Each core computes 16 image
rows (512 tokens) of one sample, reading 19 rows (3-row halo) of input.
Bottom halves are row-flipped on host so all cores run one SPMD graph.

Per-core pipeline (bf16 matmuls, f32 stats/residuals):
  LN1 (DVE, magic-rsqrt) -> PE-transpose xhat -> QKV^T matmuls ->
  S^T = K^T.T @ Q^T per (4-row group, head) with keys-on-partitions,
  exp on ACT, multiplicative exp-bias/mask tensor E (host-precomputed),
  AV with ones-augmented V (denominators land in psum rows 32/96),
  reciprocal + PE-broadcast + normalize-evac, proj + residual,
  LN2 -> MLP (gelu on ACT) -> residual -> out.
"""

import numpy as np
import ml_dtypes

KS = 7
HEADS = 8
DIM = 256
HID = 1024
HD = 32
H = 32
W = 32
NT = 19 * 32          # local tokens incl halo
NQ = 512              # query tokens per core
EPS = 1e-5
BF16 = ml_dtypes.bfloat16

# token tiles covering NT
TOK_TILES = [(0, 128), (128, 128), (256, 128), (384, 128), (512, 96)]
# groups: (key_base_token, chunk_starts)
GROUPS = [(0, (0, 96)), (32, (0, 128, 192)), (160, (0, 128, 192)), (288, (0, 128, 192))]
# distinct V key-windows (start tokens, each 128 wide)
WINDOW_STARTS = sorted({kb + cs for kb, css in GROUPS for cs in css})
WIN_IDX = {s: i for i, s in enumerate(WINDOW_STARTS)}

MAGIC = 0x5F3759DF

_CACHE = {}


def _bf(x):
    return np.ascontiguousarray(np.asarray(x, np.float32).astype(BF16))


def _f32(x):
    return np.ascontiguousarray(np.asarray(x, np.float32))


# --------------------------------------------------------------------------
# Host-side folding + mask construction
# --------------------------------------------------------------------------

def _fold_weights(inp):
    quality = inp['quality']
    s = int(quality) - 1
    l = float(quality % 1)
    g1 = np.abs(np.asarray(inp['gamma_1'], np.float64))
    g2 = np.abs(np.asarray(inp['gamma_2'], np.float64))
    if s == g1.shape[0] - 1:
        G1, G2 = g1[s], g2[s]
    else:
        G1 = g1[s] ** (1 - l) * g1[s + 1] ** l
        G2 = g2[s] ** (1 - l) * g2[s + 1] ** l

    qkv_w = np.asarray(inp['qkv_w'], np.float64)
    qkv_b = np.asarray(inp['qkv_b'], np.float64)
    n1w = np.asarray(inp['norm1_w'], np.float64)
    n1b = np.asarray(inp['norm1_b'], np.float64)
    Wq = qkv_w * n1w[None, :]
    bq = qkv_b + qkv_w @ n1b
    sc = HD ** -0.5
    Wq[:DIM] *= sc
    bq[:DIM] *= sc

    pw = G1[:, None] * np.asarray(inp['proj_w'], np.float64)
    pb = G1 * np.asarray(inp['proj_b'], np.float64)

    n2w = np.asarray(inp['norm2_w'], np.float64)
    n2b = np.asarray(inp['norm2_b'], np.float64)
    f1w = np.asarray(inp['fc1_w'], np.float64) * n2w[None, :]
    f1b = np.asarray(inp['fc1_b'], np.float64) + np.asarray(inp['fc1_w'], np.float64) @ n2b
    f2w = G2[:, None] * np.asarray(inp['fc2_w'], np.float64)
    f2b = G2 * np.asarray(inp['fc2_b'], np.float64)

    Wv = Wq[2 * DIM:]            # [256 vdims, 256 c]
    bv = bq[2 * DIM:]
    # ones-augmented V: pair p = heads (2p, 2p+1): cols 66p..66p+65 =
    # [32 dims a | one_a | 32 dims b | one_b]
    # pair p occupies V'' cols 97p..97p+96:
    # [dims_a(32) | zeros(31) | one_a | one_b | dims_b(32)]
    # head-a AV matmul: M=64 cols [0:64]  -> psum rows 0-63 (denom_a row 63)
    # head-b AV matmul: M=33 cols [64:97] -> psum rows 64-96 (denom_b row 64)
    Wv_aug = np.zeros((DIM, 388))
    vb_aug = np.zeros(388)
    for p in range(4):
        ha, hb = 2 * p, 2 * p + 1
        base = 97 * p
        Wv_aug[:, base:base + 32] = Wv[32 * ha:32 * ha + 32].T
        vb_aug[base:base + 32] = bv[32 * ha:32 * ha + 32]
        vb_aug[base + 32] = 1.0
        vb_aug[base + 64] = 1.0
        Wv_aug[:, base + 65:base + 97] = Wv[32 * hb:32 * hb + 32].T
        vb_aug[base + 65:base + 97] = bv[32 * hb:32 * hb + 32]

    # proj lhsT comes from attnT' tiles [97, 128]: pair p rows 0-32 = head 2p
    # (+denom row 32), rows 64-96 = head 2p+1. Rows 33-63 junk -> zero weight.
    # pwT_aug [4 pairs, 97, 256]
    pwT_aug = np.zeros((4, 97, DIM))
    for p in range(4):
        ha, hb = 2 * p, 2 * p + 1
        pwT_aug[p, 0:32] = pw[:, 32 * ha:32 * ha + 32].T
        pwT_aug[p, 65:97] = pw[:, 32 * hb:32 * hb + 32].T

    qkb_row = np.ascontiguousarray(bq[None, :512])
    f1b_col = np.zeros((128, 8))
    for mh in range(8):
        f1b_col[:, mh] = f1b[128 * mh:128 * mh + 128]

    # pairsel for recip broadcast: lhsT [2, 97]
    pairsel = np.zeros((1, 2 * 97))
    pairsel[0, 0:64] = 1.0
    pairsel[0, 97 + 64:97 + 97] = 1.0

    def kblocked(wT, kb):
        # [kb*128, N] -> [128, kb, N] (partition-major, contiguous for DMA)
        n = wT.shape[1]
        return np.ascontiguousarray(wT.reshape(kb, 128, n).transpose(1, 0, 2))

    # big blob [128, 1024+776+2048+2048 = 5896] bf16:
    # wqk | wv | f1w | f2w  (E blobs appended per-core later)
    big = np.concatenate([
        kblocked(Wq[:512].T, 2).reshape(128, -1),
        kblocked(Wv_aug, 2).reshape(128, -1),
        kblocked(f1w.T, 2).reshape(128, -1),
        kblocked(f2w.T, 8).reshape(128, -1),
    ], axis=1)
    # small blob [1, 512+388+256+256+194 = 1606]: qkb | vb | pb | f2b | pairsel
    small = np.concatenate([
        qkb_row, vb_aug[None, :], pb[None, :], f2b[None, :], pairsel], axis=1)

    return dict(
        wbig=_bf(big),                         # [128, 5896]
        wsmall=_bf(small),                     # [1, 1606]
        pwT=_bf(np.ascontiguousarray(pwT_aug.transpose(1, 0, 2))),  # [97, 4, 256]
        f1b=_f32(f1b_col),                     # [128, 8]
        rpb=np.asarray(inp['rpb'], np.float64),
    )


def _build_E(rpb, flip):
    """Vectorized E (exp of bias, masked/dedup-zeroed).
    Returns E_edge [8,128,256], E_std [8,128,384] float32."""
    def img_row(r):
        return (31 - r) if flip else r

    def make(group):
        if group == 0:
            keybase, chunk_starts = 0, np.array([0, 96])
        else:
            keybase, chunk_starts = (4 * group - 3) * 32, np.array([0, 128, 192])
        nch = len(chunk_starts)
        a = np.arange(4)[:, None, None, None]         # q row in group
        qj = np.arange(32)[None, :, None, None]
        c = np.arange(nch)[None, None, :, None]
        kk = np.arange(128)[None, None, None, :]
        key = chunk_starts[c] + kk                    # rel key idx
        tloc = (keybase + key) // 32
        kj = (keybase + key) % 32
        rloc_q = 4 * group + a
        qi = img_row(rloc_q)
        ki = img_row(tloc)
        sh = np.clip(qi - 3, 0, H - KS)
        sw = np.clip(qj - 3, 0, H - KS)
        valid = (ki >= sh) & (ki < sh + KS) & (kj >= sw) & (kj < sw + KS)
        if nch > 1:
            dedup = ~((c > 0) & (key < chunk_starts[np.maximum(c - 1, 0)] + 128))
            valid = valid & dedup
        bh = np.clip(ki - qi + KS - 1, 0, 2 * KS - 2)
        bw = np.clip(kj - qj + KS - 1, 0, 2 * KS - 2)
        # [8, 4, 32, nch, 128]
        bias = rpb[:, bh, bw]
        E = np.where(valid[None], np.exp(bias), 0.0)
        return np.ascontiguousarray(
            E.reshape(HEADS, 4 * 32, nch * 128).astype(np.float32))
    return make(0), make(1)


def _prepare_inputs(inp):
    F = _fold_weights(inp)
    E_e_t, E_s_t = _build_E(F['rpb'], flip=False)
    E_e_b, E_s_b = _build_E(F['rpb'], flip=True)
    x = np.asarray(inp['x'], np.float32)
    Bsz = x.shape[0]
    shared = {k: v for k, v in F.items() if k != 'rpb'}
    in_maps = []
    for b in range(Bsz):
        for half in range(2):
            if half == 0:
                x_loc = x[b, 0:19].reshape(NT, DIM)
                Ee, Es = E_e_t, E_s_t
            else:
                x_loc = x[b, 31:12:-1].reshape(NT, DIM)
                Ee, Es = E_e_b, E_s_b
            m = dict(shared)
            m['x'] = _f32(x_loc)
            # device layout: [key-within-chunk(128), head, chunk*128 + q]
            def dev(E):
                nch = E.shape[2] // 128
                return np.ascontiguousarray(
                    E.reshape(HEADS, 128, nch, 128)
                    .transpose(3, 0, 2, 1)
                    .reshape(128, HEADS * nch * 128))
            m['Eall'] = _bf(np.concatenate([dev(Ee), dev(Es)], axis=1))
            in_maps.append(m)
    return in_maps


# --------------------------------------------------------------------------
# Bass kernel graph
# --------------------------------------------------------------------------

def build_graph(debug_taps=False):
    import concourse.bass as bass
    import concourse.tile as tile
    import concourse.mybir as mybir
    from concourse import bacc
    from concourse.masks import make_identity

    dt = mybir.dt
    Alu = mybir.AluOpType
    Act = mybir.ActivationFunctionType

    nc = bacc.Bacc()

    def param(name, shape, dtype, out=False):
        return nc.declare_dram_parameter(name, list(shape), dtype, isOutput=out)

    x_d = param("x", (NT, DIM), dt.float32)
    wbig_d = param("wbig", (128, 5896), dt.bfloat16)
    wsmall_d = param("wsmall", (1, 1606), dt.bfloat16)
    pwT_d = param("pwT", (97, 4, DIM), dt.bfloat16)
    f1b_d = param("f1b", (128, 8), dt.float32)
    Eall_d = param("Eall", (128, HEADS * (256 + 384)), dt.bfloat16)
    out_d = param("out", (NQ, DIM), dt.float32, out=True)
    if debug_taps:
        dbg_qk_d = param("dbg_qk", (4, 128, NT), dt.float32, out=True)
        dbg_at_d = param("dbg_at", (4, 97, NQ), dt.float32, out=True)
        dbg_aq_d = param("dbg_aq", (6, 128, 4 * 384), dt.float32, out=True)

    with tile.TileContext(nc) as tc:
        with (
            tc.tile_pool(name="consts", bufs=1) as consts,
            tc.tile_pool(name="persist", bufs=1) as persist,
            tc.tile_pool(name="work", bufs=3) as work,
            tc.tile_pool(name="aq", bufs=3) as aqpool,
            tc.tile_pool(name="ps", bufs=7, space="PSUM") as ps,
            tc.tile_pool(name="psw", bufs=1, space="PSUM") as psw,
        ):
            # ---------------- x load (first: LN1 starts immediately) ----
            x_tiles = []
            for t, (off, nt) in enumerate(TOK_TILES):
                xt = persist.tile([128, DIM], dt.float32, tag=f"x{t}", name=f"x{t}")
                nc.sync.dma_start(out=xt[:nt], in_=x_d[off:off + nt, :])
                x_tiles.append(xt)

            # ---------------- consts ----------------
            ident = consts.tile([128, 128], dt.bfloat16, tag="ident")
            make_identity(nc, ident)

            wbig_sb = consts.tile([128, 5896], dt.bfloat16, tag="wbig")
            nc.sync.dma_start(out=wbig_sb, in_=wbig_d[:])
            wqk_sb = wbig_sb[:, 0:1024].rearrange("p (kb m) -> p kb m", kb=2)
            wv_sb = wbig_sb[:, 1024:1800].rearrange("p (kb m) -> p kb m", kb=2)
            f1w_sb = wbig_sb[:, 1800:3848].rearrange("p (kb m) -> p kb m", kb=2)
            f2w_sb = wbig_sb[:, 3848:5896].rearrange("p (kb m) -> p kb m", kb=8)

            wsm_sb = consts.tile([1, 1606], dt.bfloat16, tag="wsm")
            nc.sync.dma_start(out=wsm_sb, in_=wsmall_d[:])
            qkb_sb = wsm_sb[:, 0:512]
            vb_sb = wsm_sb[:, 512:900]
            pb_sb = wsm_sb[:, 900:1156]
            f2b_sb = wsm_sb[:, 1156:1412]
            psel_sb = wsm_sb[:, 1412:1606]

            pw_sb = consts.tile([97, 4, DIM], dt.bfloat16, tag="pw")
            nc.sync.dma_start(out=pw_sb, in_=pwT_d[:])
            f1b_sb = consts.tile([128, 8], dt.float32, tag="f1b")
            nc.sync.dma_start(out=f1b_sb, in_=f1b_d[:])
            ones_sb = consts.tile([1, NT], dt.bfloat16, tag="ones")
            nc.vector.memset(ones_sb, 1.0)
            Eall_sb = consts.tile([128, HEADS * 640], dt.bfloat16, tag="Eall")
            nc.sync.dma_start(out=Eall_sb, in_=Eall_d[:])
            Ee_sb = Eall_sb[:, 0:HEADS * 256].rearrange("p (h c) -> p h c", h=HEADS)
            Es_sb = Eall_sb[:, HEADS * 256:].rearrange("p (h c) -> p h c", h=HEADS)

            # ---------------- helpers ----------------
            def dve_rsqrt(dst, src, n):
                """dst[:,0:n] = 1/sqrt(src[:,0:n] + EPS); small-n f32 tiles."""
                ve = work.tile([128, n], dt.float32, tag="rsq_ve")
                nc.vector.tensor_scalar(out=ve, in0=src, scalar1=float(EPS),
                                        scalar2=None, op0=Alu.add)
                yi = work.tile([128, n], dt.int32, tag="rsq_yi")
                nc.vector.tensor_scalar(out=yi, in0=ve[:].bitcast(dt.int32),
                                        scalar1=1, scalar2=None,
                                        op0=Alu.logical_shift_right)
                nc.vector.tensor_scalar(out=yi, in0=yi, scalar1=-1,
                                        scalar2=MAGIC, op0=Alu.mult, op1=Alu.add)
                y = yi[:].bitcast(dt.float32)
                t = work.tile([128, n], dt.float32, tag="rsq_t")
                for _ in range(2):
                    nc.vector.tensor_tensor(out=t, in0=y, in1=y, op=Alu.mult)
                    nc.vector.tensor_tensor(out=t, in0=t, in1=ve, op=Alu.mult)
                    nc.vector.tensor_scalar(out=t, in0=t, scalar1=-0.5,
                                            scalar2=1.5, op0=Alu.mult, op1=Alu.add)
                    nc.vector.tensor_tensor(out=y, in0=y, in1=t, op=Alu.mult)
                nc.vector.tensor_copy(out=dst, in_=y)

            def layernorm_to_bf16(x_tiles, sizes, tagp):
                """Returns list of bf16 xhat tiles [nt, 256]."""
                ntile = len(x_tiles)
                mv = persist.tile([128, 2 * ntile], dt.float32, tag=tagp + "_mv")
                nc.vector.memset(mv, 1.0)
                for t, (xt, nt) in enumerate(zip(x_tiles, sizes)):
                    stats = work.tile([128, 6], dt.float32, tag=tagp + "_st")
                    nc.vector.bn_stats(out=stats[:nt], in_=xt[:nt])
                    nc.vector.bn_aggr(out=mv[:nt, 2 * t:2 * t + 2], in_=stats[:nt])
                rstd = persist.tile([128, ntile], dt.float32, tag=tagp + "_rs")
                dve_rsqrt(rstd, mv[:, 1::2], ntile)
                outs = []
                for t, (xt, nt) in enumerate(zip(x_tiles, sizes)):
                    xc = work.tile([128, DIM], dt.float32, tag=tagp + "_xc")
                    nc.vector.tensor_scalar(
                        out=xc[:nt], in0=xt[:nt],
                        scalar1=mv[:nt, 2 * t:2 * t + 1],
                        scalar2=None, op0=Alu.subtract)
                    xh = persist.tile([128, DIM], dt.bfloat16,
                                      tag=f"{tagp}_xh{t}", name=f"{tagp}_xh{t}")
                    nc.vector.tensor_scalar(
                        out=xh[:nt], in0=xc[:nt],
                        scalar1=rstd[:nt, t:t + 1],
                        scalar2=None, op0=Alu.mult)
                    outs.append((xh, nt))
                return outs

            def transpose_cat(xh_tiles, tagp, total):
                """Transpose list of [nt, 256] bf16 tiles into two [128, total]
                bf16 tiles (c-blocks)."""
                res = [persist.tile([128, total], dt.bfloat16, tag=f"{tagp}_{cb}", name=f"{tagp}_{cb}")
                       for cb in range(2)]
                off = 0
                for xh, nt in xh_tiles:
                    for cb in range(2):
                        ptb = ps.tile([128, 128], dt.bfloat16, tag="ps")
                        nc.tensor.transpose(ptb[:, :nt], xh[:nt, 128 * cb:128 * (cb + 1)],
                                            ident[:nt, :nt])
                        nc.scalar.activation(out=res[cb][:, off:off + nt],
                                             in_=ptb[:, :nt], func=Act.Copy)
                        pe_keepalive(3)
                    off += nt
                return res

            # ---------------- PE warm-up (HAM to 8/8 during LN1) --------
            pwarm = psw.tile([128, 128], dt.float32, tag="pwarm", name="pwarm")
            warm_i = [0]

            def pe_keepalive(n):
                for _ in range(n):
                    nc.tensor.matmul(pwarm[:, :128], ident, ident,
                                     start=True, stop=True)
                    warm_i[0] += 1
            pe_keepalive(30)

            # ---------------- LN1 ----------------
            sizes = [nt for _, nt in TOK_TILES]
            xh1 = layernorm_to_bf16(x_tiles, sizes, "ln1")
            xhatT = transpose_cat(xh1, "xhatT", NT)

            # ---------------- QKV ----------------
            qT, kT = [], []
            for mt in range(4):
                dst = persist.tile([128, NT if mt >= 2 else NQ], dt.bfloat16,
                                   tag=f"qk{mt}")
                ncols = NQ if mt < 2 else NT
                for n0 in range(0, ncols, 512):
                    nn = min(512, ncols - n0)
                    pt = ps.tile([128, 512], dt.float32, tag="ps")
                    for kb in range(2):
                        nc.tensor.matmul(
                            pt[:, :nn], wqk_sb[:, kb, 128 * mt:128 * (mt + 1)],
                            xhatT[kb][:, n0:n0 + nn],
                            start=(kb == 0), stop=False)
                    nc.tensor.matmul(
                        pt[:, :nn], qkb_sb[:, 128 * mt:128 * (mt + 1)],
                        ones_sb[:, :nn], start=False, stop=True)
                    nc.scalar.activation(out=dst[:, n0:n0 + nn], in_=pt[:, :nn],
                                         func=Act.Copy)
                (qT if mt < 2 else kT).append(dst)

            # V: 5 aligned token tiles, then phase-shifted windows via DMA
            Valn = []
            for t, (off, nt) in enumerate(TOK_TILES):
                vt = persist.tile([128, 388], dt.bfloat16, tag=f"va{t}", name=f"va{t}")
                pt = ps.tile([128, 512], dt.float32, tag="ps", name=f"pv{t}")
                pv = pt[:nt, :388]
                for kb in range(2):
                    nc.tensor.matmul(pv, xhatT[kb][:, off:off + nt], wv_sb[:, kb, :],
                                     start=(kb == 0), stop=False)
                nc.tensor.matmul(pv, ones_sb[:, :nt], vb_sb, start=False, stop=True)
                nc.scalar.activation(out=vt[:nt], in_=pv, func=Act.Copy)
                Valn.append(vt)
            Vw = []
            for wi, ws in enumerate(WINDOW_STARTS):
                t0, r0 = divmod(ws, 128)
                if r0 == 0:
                    Vw.append(Valn[t0])
                    continue
                vt = persist.tile([128, 388], dt.bfloat16, tag=f"vw{wi}", name=f"vw{wi}")
                n0 = 128 - r0
                nc.sync.dma_start(out=vt[:n0], in_=Valn[t0][r0:128])
                nc.sync.dma_start(out=vt[n0:128], in_=Valn[t0 + 1][:r0])
                Vw.append(vt)

            # ---------------- attention (software-pipelined groups) ------
            attnT = [persist.tile([97, NQ], dt.bfloat16, tag=f"attnT{p}", name=f"attnT{p}")
                     for p in range(4)]

            def qk_phase(g):
                kb_tok, css = GROUPS[g]
                nch = len(css)
                E_sb = Ee_sb if g == 0 else Es_sb
                aq = [aqpool.tile([128, 4, 384], dt.bfloat16, tag="aquad", name=f"aq_{g}_{q_}")
                      for q_ in range(2)]
                for quad in range(2):
                    pS_h = [ps.tile([128, 512], dt.float32, tag="ps",
                                    name=f"pS_{g}_{quad}_{s_}") for s_ in range(4)]
                    for c, cs in enumerate(css):
                        for slot in range(4):
                            nc.tensor.matmul(
                                pS_h[slot][:, 128 * c:128 * (c + 1)],
                                kT[quad][32 * slot:32 * slot + 32,
                                         kb_tok + cs:kb_tok + cs + 128],
                                qT[quad][32 * slot:32 * slot + 32,
                                         128 * g:128 * (g + 1)],
                                start=True, stop=True,
                                tile_position=(32 * slot, 0))
                    for slot in range(4):
                        nc.scalar.activation(
                            out=aq[quad][:, slot, :128 * nch],
                            in_=pS_h[slot][:, :128 * nch], func=Act.Exp)
                for quad in range(2):
                    nc.vector.tensor_tensor(
                        out=aq[quad][:, :, :128 * nch],
                        in0=aq[quad][:, :, :128 * nch],
                        in1=E_sb[:, 4 * quad:4 * quad + 4, :128 * nch],
                        op=Alu.mult)
                return aq

            def av_phase(g, aq):
                kb_tok, css = GROUPS[g]
                nch = len(css)
                for p in range(4):
                    ha, hb = 2 * p, 2 * p + 1
                    pN = ps.tile([128, 512], dt.float32, tag="ps", name=f"pN_{g}_{p}")
                    pnum = pN[:97, :128]
                    for hh, po, mm in ((ha, 0, 64), (hb, 64, 33)):
                        quad, slot = divmod(hh, 4)
                        voff = 97 * p + (0 if po == 0 else 64)
                        for c, cs in enumerate(css):
                            vt = Vw[WIN_IDX[kb_tok + cs]]
                            nc.tensor.matmul(
                                pnum[po:po + mm, :],
                                vt[:, voff:voff + mm],
                                aq[quad][:, slot, 128 * c:128 * (c + 1)],
                                start=(c == 0), stop=(c == nch - 1),
                                tile_position=(0, po))
                    den = work.tile([1, 256], dt.float32, tag="den")
                    nc.vector.tensor_copy(out=den[:, 0:128], in_=pN[32:33, :128])
                    nc.vector.tensor_copy(out=den[:, 128:256], in_=pN[64:65, :128])
                    numsb = work.tile([97, 128], dt.bfloat16, tag="numsb")
                    nc.vector.tensor_copy(out=numsb, in_=pnum)
                    rcd = work.tile([1, 256], dt.float32, tag="rcd")
                    nc.vector.reciprocal_approx_fast(out=rcd, in_=den)
                    rcb16 = work.tile([1, 256], dt.bfloat16, tag="rcb16")
                    nc.vector.tensor_copy(out=rcb16, in_=rcd)
                    rc0 = rcb16[:, 0:128]
                    rc1 = rcb16[:, 128:256]
                    pB = ps.tile([128, 512], dt.float32, tag="ps", name=f"pB_{g}_{p}")
                    nc.tensor.matmul(pB[:97, :128], psel_sb[:, 0:97], rc0,
                                     start=True, stop=False)
                    nc.tensor.matmul(pB[:97, :128], psel_sb[:, 97:194], rc1,
                                     start=False, stop=True)
                    rcb = work.tile([97, 128], dt.bfloat16, tag="rcb")
                    nc.vector.tensor_copy(out=rcb, in_=pB[:97, :128])
                    nc.vector.tensor_tensor(
                        out=attnT[p][:, 128 * g:128 * (g + 1)],
                        in0=numsb, in1=rcb, op=Alu.mult)

            prev = None
            for g in range(4):
                aq = qk_phase(g)
                if prev is not None:
                    av_phase(prev[0], prev[1])
                prev = (g, aq)
            av_phase(prev[0], prev[1])

            # ---------------- proj + residual ----------------
            y_tiles = []
            for mt in range(4):
                pt = ps.tile([128, 512], dt.float32, tag="ps")
                pp = pt[:, :DIM]
                for p in range(4):
                    nc.tensor.matmul(pp, attnT[p][:, 128 * mt:128 * (mt + 1)],
                                     pw_sb[:, p, :], start=(p == 0), stop=False)
                nc.tensor.matmul(pp, ones_sb[:, :128], pb_sb, start=False, stop=True)
                yt = persist.tile([128, DIM], dt.float32, tag=f"y{mt}")
                nc.vector.tensor_tensor(out=yt, in0=pp, in1=x_tiles[mt][:],
                                        op=Alu.add)
                y_tiles.append(yt)

            # ---------------- LN2 + MLP ----------------
            xh2 = layernorm_to_bf16(y_tiles, [128] * 4, "ln2")
            x2T = transpose_cat(xh2, "x2T", NQ)

            m1 = []
            for mh in range(8):
                pt = ps.tile([128, 512], dt.float32, tag="ps")
                for kb in range(2):
                    nc.tensor.matmul(pt, f1w_sb[:, kb, 128 * mh:128 * (mh + 1)],
                                     x2T[kb], start=(kb == 0), stop=(kb == 1))
                mg = persist.tile([128, NQ], dt.bfloat16, tag=f"m1_{mh}")
                nc.scalar.activation(out=mg, in_=pt, func=Act.Gelu,
                                     bias=f1b_sb[:, mh:mh + 1], scale=1.0)
                m1.append(mg)

            pt2 = [ps.tile([128, 512], dt.float32, tag="ps", name=f"fc2_{mt}")
                   for mt in range(4)]
            for kb in range(8):
                for mt in range(4):
                    nc.tensor.matmul(pt2[mt][:, :DIM],
                                     m1[kb][:, 128 * mt:128 * (mt + 1)],
                                     f2w_sb[:, kb, :], start=(kb == 0), stop=False)
            for mt in range(4):
                pp = pt2[mt][:, :DIM]
                nc.tensor.matmul(pp, ones_sb[:, :128], f2b_sb, start=False, stop=True)
                ot = work.tile([128, DIM], dt.float32, tag="outt")
                nc.vector.tensor_tensor(out=ot, in0=pp, in1=y_tiles[mt][:],
                                        op=Alu.add)
                nc.sync.dma_start(out=out_d[128 * mt:128 * (mt + 1), :], in_=ot)

    nc.finalize()
    return nc


# --------------------------------------------------------------------------
# Entry point
# --------------------------------------------------------------------------

def kernel(**inputs):
    from concourse.bass_utils import run_bass_kernel_spmd

    if 'nc' not in _CACHE:
        _CACHE['nc'] = build_graph()
    nc = _CACHE['nc']

    in_maps = _prepare_inputs(inputs)
    res = run_bass_kernel_spmd(nc, in_maps, core_ids=list(range(8)))
    x = np.asarray(inputs['x'])
    Bsz, Hh, Ww, C = x.shape
    out = np.zeros((Bsz, Hh, Ww, C), np.float32)
    for i in range(2 * Bsz):
        b, half = divmod(i, 2)
        o = np.asarray(res.results[i]['out']).reshape(16, Ww, C)
        if half == 0:
            out[b, 0:16] = o
        else:
            out[b, 16:32] = o[::-1]
    return out.astype(x.dtype)

